# revision 2
# baseline (speedup 1.0000x reference)
"""Block 8x8 DCT kernel for Trainium2 (Bass/Tile), 8-core data-parallel.

Full input x [32, 3, 1024, 1024] fp32 -> output [32, 192, 128, 128] fp32.
Sharded batch-wise: each of the 8 cores processes [4, 3, 1024, 1024].

Algorithm per core, per [128-row x 1024-col] band of one (b, c) image:
  - The band is 16 block-rows (hb) x 8 rows-in-block (r) on partitions,
    128 blocks (w) x 8 cols-in-block (s) in the free dim.
  - Pass 1: for each 128-wide free chunk, matmul with the DATA as the
    stationary operand (lhsT) and a constant K = kron(I16, A.T) as the
    moving operand: out[wl*8+s, hb*8+u] = sum_r A[u,r] * x[hb*8+r, ...].
    This contracts r (row DCT) and transposes the chunk in one PE op.
  - Copy PSUM -> SBUF (ScalarE).
  - Pass 2: same trick again with the same constant K: contracts s
    (col DCT) and transposes back: out[hb*8+u, wl*8+v].
  - Copy PSUM -> SBUF with a free-dim shuffle (DVE) so the DMA-out has
    512B-contiguous DRAM runs: free (c16,wl,v) -> (v, w=16c+wl).
  - DMA out to y[b, cimg*64+u*8+v, band*16+hb, w].
"""

import numpy as np

N = 8
PI = 3.1415  # matches reference (not math.pi)

_B_FULL = 32
_C = 3
_H = 1024
_W = 1024
_NCORES = 8
_B_CORE = _B_FULL // _NCORES


def _dct_basis_np():
    x = np.arange(N, dtype=np.float32)
    freqs = ((2.0 * x + 1.0) / (2.0 * N) * np.float32(PI)).astype(np.float32)
    return np.cos(freqs[:, None] * x[None, :]).astype(np.float32)  # A[u, r]


def _const_k():
    # K[g*8 + r, g*8 + u] = A[u, r] for g in 0..15  (block-diag kron(I16, A.T))
    A = _dct_basis_np()
    return np.kron(np.eye(16, dtype=np.float32), A.T).astype(np.float32)


def build_nc(B, C, H, W):
    import concourse.bass as bass  # noqa: F401
    import concourse.mybir as mybir
    import concourse.tile as tile
    from concourse import bacc

    f32 = mybir.dt.float32
    nbands = H // 128
    assert H % 128 == 0 and W == 1024

    nc = bacc.Bacc("TRN2", target_bir_lowering=False, debug=False,
                   num_devices=_NCORES)
    x = nc.dram_tensor("x", [B, C, H, W], f32, kind="ExternalInput").ap()
    w = nc.dram_tensor("w", [128, 128], f32, kind="ExternalInput").ap()
    y = nc.dram_tensor("y", [B, C * 64, H // 8, W // 8], f32,
                       kind="ExternalOutput").ap()

    # y viewed as [b, cimg, band, hb, u, v, w]
    yv = y.rearrange("bb (ci u v) (bd hb) w -> bb ci bd hb u v w",
                     u=8, v=8, hb=16)

    with tile.TileContext(nc) as tc:
        with (
            tc.tile_pool(name="const", bufs=1) as constp,
            tc.tile_pool(name="xin", bufs=3) as xp,
            tc.tile_pool(name="z", bufs=2) as zp,
            tc.tile_pool(name="o", bufs=3) as op_,
            tc.tile_pool(name="ps1", bufs=2, space="PSUM") as ps1p,
            tc.tile_pool(name="ps2", bufs=2, space="PSUM") as ps2p,
        ):
            wt = constp.tile([128, 128], f32)
            nc.sync.dma_start(wt[:], w[:])
            for b in range(B):
                for c in range(C):
                    for band in range(nbands):
                        xt = xp.tile([128, 1024], f32)
                        nc.sync.dma_start(
                            xt[:], x[b, c, band * 128:(band + 1) * 128, :])

                        ps1 = ps1p.tile([128, 1024], f32)
                        for cc in range(8):
                            nc.tensor.matmul(
                                ps1[:, cc * 128:(cc + 1) * 128],
                                xt[:, cc * 128:(cc + 1) * 128],
                                wt[:],
                            )
                        zt = zp.tile([128, 1024], f32)
                        nc.scalar.copy(zt[:], ps1[:])

                        ps2 = ps2p.tile([128, 1024], f32)
                        for cc in range(8):
                            nc.tensor.matmul(
                                ps2[:, cc * 128:(cc + 1) * 128],
                                zt[:, cc * 128:(cc + 1) * 128],
                                wt[:],
                            )
                        ot = op_.tile([128, 1024], f32)
                        # free shuffle: (c16, wl, v) -> (v, c16, wl)
                        nc.vector.tensor_copy(
                            ot[:].rearrange("p (v c w) -> p c w v",
                                            v=8, c=8, w=16),
                            ps2[:].rearrange("p (c w v) -> p c w v",
                                             c=8, w=16, v=8),
                        )
                        # ot enumerates (hb,u,v,w) in plain (p, f) order, so
                        # the 2D AP matches yv's 4-dim AP element order.
                        nc.sync.dma_start(yv[b, c, band], ot[:])
    nc.compile()
    return nc


_NC_CACHE = {}


def _get_nc(B, C, H, W):
    key = (B, C, H, W)
    if key not in _NC_CACHE:
        _NC_CACHE[key] = build_nc(B, C, H, W)
    return _NC_CACHE[key]


def kernel(x: np.ndarray) -> np.ndarray:
    from concourse import bass_utils

    x = np.ascontiguousarray(x, dtype=np.float32)
    assert x.shape == (_B_FULL, _C, _H, _W), x.shape

    nc = _get_nc(_B_CORE, _C, _H, _W)
    K = _const_k()
    in_maps = [
        {"x": np.ascontiguousarray(x[i * _B_CORE:(i + 1) * _B_CORE]), "w": K}
        for i in range(_NCORES)
    ]
    res = bass_utils.run_bass_kernel_spmd(
        nc, in_maps, core_ids=list(range(_NCORES)))
    out = np.concatenate([r["y"] for r in res.results], axis=0)
    return out
